# revision 4
# baseline (speedup 1.0000x reference)
"""Trainium2 Bass kernel for an 8-expert top-2 MoE layer (B=4, T=2048, C=1024,
F=4096), expert-parallel across 8 NeuronCores.

Strategy
--------
The reference module is a *dense* MoE: it runs every expert's FFN on every
token, then combines with top-2 gate weights — so 6 of 8 expert outputs per
token are multiplied by zero.  The output only depends on each token's top-2
experts, so we route: the host computes the (tiny) gate in fp32, assigns each
token to its two experts, and each NeuronCore runs one expert's FFN over just
the tokens routed to it (~2*BT/E tokens, padded to a common multiple of 512).
The host then scatter-adds the gate-weighted per-expert outputs.

The gate MUST be computed in fp32: the smallest 2nd-vs-3rd expert logit margin
over the 8192 tokens is ~3.6e-5, and a bf16 gate flips the selected expert set
for ~17 tokens, each flip producing an O(1) relative error at that token.  The
fp32 host gate matches the reference selection with a ~20x margin.

On-device math per core (expert e):
    hT[f, t]   = sum_c W1[c, f] * xT[c, t]        (PE, bf16 inputs, fp32 acc)
    hT         = gelu_erf(hT + b1)                (ScalarE, fused bias)
    out[t, cc] = sum_f h[t, f] * W2[f, cc]        (PE, bf16 h, fp32 acc)
    out        = out + b2                         (VectorE, fp32)
Computing h in transposed form (tokens in the free dim) is what lets the
second matmul contract over F without any on-device transpose.
"""

import os

import numpy as np
import ml_dtypes

import concourse.bass as bass
import concourse.mybir as mybir
import concourse.tile as tile
from concourse import bacc
from concourse.bass_utils import run_bass_kernel_spmd

C = 1024
F = 4096
E = 8
K = 2
N_CORES = 8
CHUNK = 512  # tokens per device-side pipeline chunk

BF16 = mybir.dt.bfloat16
F32 = mybir.dt.float32


def build_nc(ntok: int) -> bass.Bass:
    """Bass program for one expert's FFN over ntok (multiple of 512) tokens."""
    assert ntok % CHUNK == 0
    n_chunks = ntok // CHUNK
    nc = bacc.Bacc(None)

    xt = nc.dram_tensor("xt", [C, ntok], BF16, kind="ExternalInput")
    w1 = nc.dram_tensor("w1", [C, F], BF16, kind="ExternalInput")
    w2 = nc.dram_tensor("w2", [F, C], BF16, kind="ExternalInput")
    b1t = nc.dram_tensor("b1t", [128, F // 128], F32, kind="ExternalInput")
    b2b = nc.dram_tensor("b2b", [128, C], F32, kind="ExternalInput")
    out = nc.dram_tensor("out", [ntok, C], F32, kind="ExternalOutput")

    n_ct = C // 128  # 8 contraction tiles for x @ W1
    n_ft = F // 128  # 32 F tiles / contraction tiles for h @ W2
    n_tt = CHUNK // 128  # 4 token tiles per chunk
    n_cc = C // 512  # 2 output column chunks

    with tile.TileContext(nc) as tc:
        with (
            tc.tile_pool(name="wpool", bufs=1) as wpool,
            tc.tile_pool(name="xpool", bufs=2) as xpool,
            tc.tile_pool(name="hpool", bufs=n_ft + 2) as hpool,
            tc.tile_pool(name="opool", bufs=4) as opool,
            tc.tile_pool(name="phpool", bufs=2, space="PSUM") as phpool,
            tc.tile_pool(name="popool", bufs=3, space="PSUM") as popool,
        ):
            w1_sb = []
            for c in range(n_ct):
                t = wpool.tile([128, F], BF16, name=f"w1sb{c}", tag=f"w1sb{c}")
                nc.sync.dma_start(out=t, in_=w1[c * 128 : (c + 1) * 128, :])
                w1_sb.append(t)
            w2_sb = []
            for f in range(n_ft):
                t = wpool.tile([128, C], BF16, name=f"w2sb{f}", tag=f"w2sb{f}")
                nc.sync.dma_start(out=t, in_=w2[f * 128 : (f + 1) * 128, :])
                w2_sb.append(t)
            b1_sb = wpool.tile([128, F // 128], F32, name="b1sb", tag="b1sb")
            nc.sync.dma_start(out=b1_sb, in_=b1t[:, :])
            b2_sb = wpool.tile([128, C], F32, name="b2sb", tag="b2sb")
            nc.sync.dma_start(out=b2_sb, in_=b2b[:, :])

            for tk in range(n_chunks):
                xts = []
                for c in range(n_ct):
                    t = xpool.tile([128, CHUNK], BF16, name=f"xt_{tk}_{c}", tag=f"xt{c}")
                    nc.sync.dma_start(
                        out=t,
                        in_=xt[c * 128 : (c + 1) * 128, tk * CHUNK : (tk + 1) * CHUNK],
                    )
                    xts.append(t)

                hts = []
                for f in range(n_ft):
                    ph = phpool.tile([128, CHUNK], F32, name=f"ph_{tk}_{f}", tag="ph")
                    for c in range(n_ct):
                        nc.tensor.matmul(
                            ph,
                            lhsT=w1_sb[c][:, f * 128 : (f + 1) * 128],
                            rhs=xts[c],
                            start=(c == 0),
                            stop=(c == n_ct - 1),
                        )
                    ht = hpool.tile([128, CHUNK], BF16, name=f"ht_{tk}_{f}", tag="ht")
                    nc.scalar.activation(
                        out=ht,
                        in_=ph,
                        func=mybir.ActivationFunctionType.Gelu,
                        bias=b1_sb[:, f : f + 1],
                        scale=1.0,
                    )
                    hts.append(ht)

                for tt in range(n_tt):
                    for cc in range(n_cc):
                        po = popool.tile(
                            [128, 512], F32, name=f"po_{tk}_{tt}_{cc}", tag="po"
                        )
                        for f in range(n_ft):
                            nc.tensor.matmul(
                                po,
                                lhsT=hts[f][:, tt * 128 : (tt + 1) * 128],
                                rhs=w2_sb[f][:, cc * 512 : (cc + 1) * 512],
                                start=(f == 0),
                                stop=(f == n_ft - 1),
                            )
                        ot = opool.tile(
                            [128, 512], F32, name=f"ot_{tk}_{tt}_{cc}", tag="ot"
                        )
                        nc.vector.tensor_add(ot, po, b2_sb[:, cc * 512 : (cc + 1) * 512])
                        r0 = tk * CHUNK + tt * 128
                        nc.sync.dma_start(
                            out=out[r0 : r0 + 128, cc * 512 : (cc + 1) * 512], in_=ot
                        )
    nc.finalize()
    return nc


def _route(x2d: np.ndarray, Wg: np.ndarray):
    """fp32 gate identical in selection to the reference; returns per-expert
    token indices and renormalized top-2 weights."""
    logits = x2d @ Wg  # fp32 BLAS
    order = np.argsort(-logits, axis=1, kind="stable")
    top2 = order[:, :K]  # [N, 2]
    m = logits.max(axis=1, keepdims=True)
    p = np.exp(logits - m, dtype=np.float32)
    p /= p.sum(axis=1, keepdims=True)
    tw = np.take_along_axis(p, top2, axis=1)
    tw /= tw.sum(axis=1, keepdims=True)  # [N, 2] renormalized
    idxs, ws = [], []
    for e in range(E):
        sel = top2 == e  # [N, 2] bool, at most one True per row
        rows = np.where(sel.any(axis=1))[0]
        idxs.append(rows)
        ws.append(tw[rows][sel[rows]])
    return idxs, ws


_LAST_RESULTS = {}  # stash for test harness introspection (exec time etc.)


def kernel(**inputs: np.ndarray) -> np.ndarray:
    x = np.asarray(inputs["x"], dtype=np.float32)
    Wg = np.asarray(inputs["Wg"], dtype=np.float32)
    W1 = np.asarray(inputs["W1"], dtype=np.float32)
    b1 = np.asarray(inputs["b1"], dtype=np.float32)
    W2 = np.asarray(inputs["W2"], dtype=np.float32)
    b2 = np.asarray(inputs["b2"], dtype=np.float32)

    B, T, Cx = x.shape
    assert Cx == C
    x2d = np.ascontiguousarray(x.reshape(-1, C))
    n_tok_total = x2d.shape[0]

    idxs, ws = _route(x2d, Wg)
    max_n = max(len(i) for i in idxs)
    ntok = ((max_n + CHUNK - 1) // CHUNK) * CHUNK

    in_maps = []
    for e in range(E):
        n_e = len(idxs[e])
        xe = np.zeros((ntok, C), dtype=np.float32)
        xe[:n_e] = x2d[idxs[e]]
        in_maps.append(
            {
                "xt": np.ascontiguousarray(xe.T).astype(ml_dtypes.bfloat16),
                "w1": W1[e].astype(ml_dtypes.bfloat16),
                "w2": W2[e].astype(ml_dtypes.bfloat16),
                "b1t": np.ascontiguousarray(
                    b1[e].reshape(F // 128, 128).T
                ).astype(np.float32),
                "b2b": np.broadcast_to(b2[e], (128, C)).copy().astype(np.float32),
            }
        )

    nc = build_nc(ntok)
    trace = os.environ.get("KERNEL_TRACE", "") == "1"
    res = run_bass_kernel_spmd(
        nc, in_maps, core_ids=list(range(N_CORES)), trace=trace
    )
    _LAST_RESULTS["bass_results"] = res
    if trace and res.exec_time_ns is not None:
        print(f"[kernel] HW exec time: {res.exec_time_ns} ns")

    out = np.zeros((n_tok_total, C), dtype=np.float32)
    for e in range(E):
        n_e = len(idxs[e])
        oe = np.asarray(res.results[e]["out"])[:n_e]
        out[idxs[e]] += ws[e][:, None] * oe
    return out.reshape(B, T, C)


# revision 7
# speedup vs baseline: 1.1319x; 1.1319x over previous
"""Trainium2 Bass kernel for an 8-expert top-2 MoE layer (B=4, T=2048, C=1024,
F=4096), expert-parallel across 8 NeuronCores.

Strategy
--------
The reference module is a *dense* MoE: it runs every expert's FFN on every
token, then combines with top-2 gate weights — so 6 of 8 expert outputs per
token are multiplied by zero.  The output only depends on each token's top-2
experts, so we route: the host computes the (tiny) gate in fp32, assigns each
token to its two experts, and each NeuronCore runs one expert's FFN over just
the tokens routed to it (~2*BT/E tokens, padded to a common multiple of 512).
The host then scatter-adds the gate-weighted per-expert outputs.

The gate MUST be computed in fp32: the smallest 2nd-vs-3rd expert logit margin
over the 8192 tokens is ~3.6e-5, and a bf16 gate flips the selected expert set
for ~17 tokens, each flip producing an O(1) relative error at that token.  The
fp32 host gate matches the reference selection with a ~20x margin.

On-device math per core (expert e):
    hT[f, t]   = sum_c W1[c, f] * xT[c, t]        (PE, bf16 inputs, fp32 acc)
    hT         = gelu_erf(hT + b1)                (ScalarE, fused bias)
    out[t, cc] = sum_f h[t, f] * W2[f, cc]        (PE, bf16 h, fp32 acc)
    out        = out + b2                         (VectorE, fp32)
Computing h in transposed form (tokens in the free dim) is what lets the
second matmul contract over F without any on-device transpose.
"""

import os

import numpy as np
import ml_dtypes

import concourse.bass as bass
import concourse.mybir as mybir
import concourse.tile as tile
from concourse import bacc
from concourse.bass_utils import run_bass_kernel_spmd

C = 1024
F = 4096
E = 8
K = 2
N_CORES = 8
CHUNK = 512  # tokens per device-side pipeline chunk

BF16 = mybir.dt.bfloat16
F32 = mybir.dt.float32


def build_nc(chunks: list[int]) -> bass.Bass:
    """Bass program for one expert's FFN over sum(chunks) tokens.

    chunks: per-pipeline-chunk token counts, each a multiple of 128, <= 512.
    """
    ntok = sum(chunks)
    assert all(ch % 128 == 0 and 0 < ch <= 512 for ch in chunks)
    nc = bacc.Bacc(None)

    xt = nc.dram_tensor("xt", [C, ntok], BF16, kind="ExternalInput")
    w1 = nc.dram_tensor("w1", [C, F], BF16, kind="ExternalInput")
    w2 = nc.dram_tensor("w2", [F, C], BF16, kind="ExternalInput")
    b1t = nc.dram_tensor("b1t", [128, F // 128], F32, kind="ExternalInput")
    b2b = nc.dram_tensor("b2b", [128, C], F32, kind="ExternalInput")
    out = nc.dram_tensor("out", [ntok, C], F32, kind="ExternalOutput")

    n_ct = C // 128  # 8 contraction tiles for x @ W1
    n_ft = F // 128  # 32 F tiles / contraction tiles for h @ W2
    n_cc = C // 512  # 2 output column chunks

    with tile.TileContext(nc) as tc:
        with (
            tc.tile_pool(name="wpool", bufs=1) as wpool,
            tc.tile_pool(name="xpool", bufs=2) as xpool,
            tc.tile_pool(name="hpool", bufs=n_ft + 2) as hpool,
            tc.tile_pool(name="opool", bufs=4) as opool,
            tc.tile_pool(name="phpool", bufs=2, space="PSUM") as phpool,
            tc.tile_pool(name="popool", bufs=3, space="PSUM") as popool,
        ):
            # Chunk-0 activations first so the PE can start ASAP, then W1 (in
            # halves, first halves first — matmul f-tile 0 only needs the
            # first half), then W2 / biases which are needed later.
            xts0 = []
            for c in range(n_ct):
                t = xpool.tile([128, chunks[0]], BF16, name=f"xt_0_{c}", tag=f"xt{c}")
                nc.sync.dma_start(out=t, in_=xt[c * 128 : (c + 1) * 128, : chunks[0]])
                xts0.append(t)
            w1_sb = []
            for c in range(n_ct):
                t = wpool.tile([128, F], BF16, name=f"w1sb{c}", tag=f"w1sb{c}")
                w1_sb.append(t)
            for half in range(2):
                fs = slice(half * (F // 2), (half + 1) * (F // 2))
                for c in range(n_ct):
                    nc.sync.dma_start(
                        out=w1_sb[c][:, fs], in_=w1[c * 128 : (c + 1) * 128, fs]
                    )
            b1_sb = wpool.tile([128, F // 128], F32, name="b1sb", tag="b1sb")
            nc.sync.dma_start(out=b1_sb, in_=b1t[:, :])
            b2_sb = wpool.tile([128, C], F32, name="b2sb", tag="b2sb")
            nc.sync.dma_start(out=b2_sb, in_=b2b[:, :])
            w2_sb = []
            for f in range(n_ft):
                t = wpool.tile([128, C], BF16, name=f"w2sb{f}", tag=f"w2sb{f}")
                nc.sync.dma_start(out=t, in_=w2[f * 128 : (f + 1) * 128, :])
                w2_sb.append(t)

            tok0 = 0
            for tk, ch in enumerate(chunks):
                if tk == 0:
                    xts = xts0
                else:
                    xts = []
                    for c in range(n_ct):
                        t = xpool.tile([128, ch], BF16, name=f"xt_{tk}_{c}", tag=f"xt{c}")
                        nc.sync.dma_start(
                            out=t,
                            in_=xt[c * 128 : (c + 1) * 128, tok0 : tok0 + ch],
                        )
                        xts.append(t)

                hts = []
                for f in range(n_ft):
                    ph = phpool.tile([128, ch], F32, name=f"ph_{tk}_{f}", tag="ph")
                    for c in range(n_ct):
                        nc.tensor.matmul(
                            ph,
                            lhsT=w1_sb[c][:, f * 128 : (f + 1) * 128],
                            rhs=xts[c],
                            start=(c == 0),
                            stop=(c == n_ct - 1),
                        )
                    ht = hpool.tile([128, ch], BF16, name=f"ht_{tk}_{f}", tag="ht")
                    nc.scalar.activation(
                        out=ht,
                        in_=ph,
                        func=mybir.ActivationFunctionType.Gelu,
                        bias=b1_sb[:, f : f + 1],
                        scale=1.0,
                    )
                    hts.append(ht)

                for tt in range(ch // 128):
                    for cc in range(n_cc):
                        po = popool.tile(
                            [128, 512], F32, name=f"po_{tk}_{tt}_{cc}", tag="po"
                        )
                        for f in range(n_ft):
                            nc.tensor.matmul(
                                po,
                                lhsT=hts[f][:, tt * 128 : (tt + 1) * 128],
                                rhs=w2_sb[f][:, cc * 512 : (cc + 1) * 512],
                                start=(f == 0),
                                stop=(f == n_ft - 1),
                            )
                        ot = opool.tile(
                            [128, 512], F32, name=f"ot_{tk}_{tt}_{cc}", tag="ot"
                        )
                        nc.vector.tensor_add(ot, po, b2_sb[:, cc * 512 : (cc + 1) * 512])
                        r0 = tok0 + tt * 128
                        nc.sync.dma_start(
                            out=out[r0 : r0 + 128, cc * 512 : (cc + 1) * 512], in_=ot
                        )
                tok0 += ch
    nc.finalize()
    return nc


def pick_chunks(max_n: int) -> list[int]:
    """Smallest [512]*a + [256 or 384 or 128]? schedule covering max_n tokens.

    Keep most chunks at 512 (best PE efficiency); one smaller tail chunk
    (multiple of 128) trims padding waste.
    """
    n512 = max_n // 512
    rem = max_n - n512 * 512
    chunks = [512] * n512
    if rem > 0:
        chunks.append(((rem + 127) // 128) * 128)
    if not chunks:
        chunks = [128]
    return chunks


def _route(x2d: np.ndarray, Wg: np.ndarray):
    """fp32 gate identical in selection to the reference; returns per-expert
    token indices and renormalized top-2 weights."""
    logits = x2d @ Wg  # fp32 BLAS
    order = np.argsort(-logits, axis=1, kind="stable")
    top2 = order[:, :K]  # [N, 2]
    m = logits.max(axis=1, keepdims=True)
    p = np.exp(logits - m, dtype=np.float32)
    p /= p.sum(axis=1, keepdims=True)
    tw = np.take_along_axis(p, top2, axis=1)
    tw /= tw.sum(axis=1, keepdims=True)  # [N, 2] renormalized
    idxs, ws = [], []
    for e in range(E):
        sel = top2 == e  # [N, 2] bool, at most one True per row
        rows = np.where(sel.any(axis=1))[0]
        idxs.append(rows)
        ws.append(tw[rows][sel[rows]])
    return idxs, ws


_LAST_RESULTS = {}  # stash for test harness introspection (exec time etc.)


def kernel(**inputs: np.ndarray) -> np.ndarray:
    x = np.asarray(inputs["x"], dtype=np.float32)
    Wg = np.asarray(inputs["Wg"], dtype=np.float32)
    W1 = np.asarray(inputs["W1"], dtype=np.float32)
    b1 = np.asarray(inputs["b1"], dtype=np.float32)
    W2 = np.asarray(inputs["W2"], dtype=np.float32)
    b2 = np.asarray(inputs["b2"], dtype=np.float32)

    B, T, Cx = x.shape
    assert Cx == C
    x2d = np.ascontiguousarray(x.reshape(-1, C))
    n_tok_total = x2d.shape[0]

    idxs, ws = _route(x2d, Wg)
    max_n = max(len(i) for i in idxs)
    chunks = pick_chunks(max_n)
    ntok = sum(chunks)

    in_maps = []
    for e in range(E):
        n_e = len(idxs[e])
        xe = np.zeros((ntok, C), dtype=np.float32)
        xe[:n_e] = x2d[idxs[e]]
        in_maps.append(
            {
                "xt": np.ascontiguousarray(xe.T).astype(ml_dtypes.bfloat16),
                "w1": W1[e].astype(ml_dtypes.bfloat16),
                "w2": W2[e].astype(ml_dtypes.bfloat16),
                "b1t": np.ascontiguousarray(
                    b1[e].reshape(F // 128, 128).T
                ).astype(np.float32),
                "b2b": np.broadcast_to(b2[e], (128, C)).copy().astype(np.float32),
            }
        )

    nc = build_nc(chunks)
    trace = os.environ.get("KERNEL_TRACE", "") == "1"
    res = run_bass_kernel_spmd(
        nc, in_maps, core_ids=list(range(N_CORES)), trace=trace
    )
    _LAST_RESULTS["bass_results"] = res
    if trace and res.exec_time_ns is not None:
        print(f"[kernel] HW exec time: {res.exec_time_ns} ns")

    out = np.zeros((n_tok_total, C), dtype=np.float32)
    for e in range(E):
        n_e = len(idxs[e])
        oe = np.asarray(res.results[e]["out"])[:n_e]
        out[idxs[e]] += ws[e][:, None] * oe
    return out.reshape(B, T, C)


# revision 11
# speedup vs baseline: 1.1522x; 1.0180x over previous
"""Trainium2 Bass kernel for an 8-expert top-2 MoE layer (B=4, T=2048, C=1024,
F=4096), expert-parallel across 8 NeuronCores.

Strategy
--------
The reference module is a *dense* MoE: it runs every expert's FFN on every
token, then combines with top-2 gate weights — so 6 of 8 expert outputs per
token are multiplied by zero.  The output only depends on each token's top-2
experts, so we route: the host computes the (tiny) gate in fp32, assigns each
token to its two experts, and each NeuronCore runs one expert's FFN over just
the tokens routed to it (~2*BT/E tokens, padded to a common multiple of 512).
The host then scatter-adds the gate-weighted per-expert outputs.

The gate MUST be computed in fp32: the smallest 2nd-vs-3rd expert logit margin
over the 8192 tokens is ~3.6e-5, and a bf16 gate flips the selected expert set
for ~17 tokens, each flip producing an O(1) relative error at that token.  The
fp32 host gate matches the reference selection with a ~20x margin.

On-device math per core (expert e):
    hT[f, t]   = sum_c W1[c, f] * xT[c, t]        (PE, bf16 inputs, fp32 acc)
    hT         = gelu_erf(hT + b1)                (ScalarE, fused bias)
    out[t, cc] = sum_f h[t, f] * W2[f, cc]        (PE, bf16 h, fp32 acc)
    out        = out + b2                         (VectorE, fp32)
Computing h in transposed form (tokens in the free dim) is what lets the
second matmul contract over F without any on-device transpose.
"""

import os

import numpy as np
import ml_dtypes

import concourse.bass as bass
import concourse.mybir as mybir
import concourse.tile as tile
from concourse import bacc
from concourse.bass_utils import run_bass_kernel_spmd

C = 1024
F = 4096
E = 8
K = 2
N_CORES = 8
CHUNK = 512  # tokens per device-side pipeline chunk

BF16 = mybir.dt.bfloat16
F32 = mybir.dt.float32


def build_nc(chunks: list[int]) -> bass.Bass:
    """Bass program for one expert's FFN over sum(chunks) tokens.

    chunks: per-pipeline-chunk token counts, each a multiple of 128, <= 512.
    """
    ntok = sum(chunks)
    assert all(0 < ch <= 512 for ch in chunks)
    nc = bacc.Bacc(None)

    xt = nc.dram_tensor("xt", [C, ntok], BF16, kind="ExternalInput")
    w1 = nc.dram_tensor("w1", [C, F], BF16, kind="ExternalInput")
    w2 = nc.dram_tensor("w2", [F, C], BF16, kind="ExternalInput")
    b1t = nc.dram_tensor("b1t", [128, F // 128], F32, kind="ExternalInput")
    b2b = nc.dram_tensor("b2b", [128, C], F32, kind="ExternalInput")
    out = nc.dram_tensor("out", [ntok, C], F32, kind="ExternalOutput")

    n_ct = C // 128  # 8 contraction tiles for x @ W1
    n_ft = F // 128  # 32 F tiles / contraction tiles for h @ W2
    n_cc = C // 512  # 2 output column chunks

    with tile.TileContext(nc) as tc:
        with (
            tc.tile_pool(name="wpool", bufs=1) as wpool,
            tc.tile_pool(name="xpool", bufs=2) as xpool,
            tc.tile_pool(name="hpool", bufs=n_ft + 2) as hpool,
            tc.tile_pool(name="opool", bufs=4) as opool,
            tc.tile_pool(name="phpool", bufs=2, space="PSUM") as phpool,
            tc.tile_pool(name="popool", bufs=3, space="PSUM") as popool,
        ):
            # Chunk-0 activations first so the PE can start ASAP, then W1 (in
            # halves, first halves first — matmul f-tile 0 only needs the
            # first half), then W2 / biases which are needed later.
            xts0 = []
            for c in range(n_ct):
                t = xpool.tile([128, chunks[0]], BF16, name=f"xt_0_{c}", tag=f"xt{c}")
                nc.sync.dma_start(out=t, in_=xt[c * 128 : (c + 1) * 128, : chunks[0]])
                xts0.append(t)
            w1_sb = []
            for c in range(n_ct):
                t = wpool.tile([128, F], BF16, name=f"w1sb{c}", tag=f"w1sb{c}")
                w1_sb.append(t)
            for quarter in range(4):
                fs = slice(quarter * (F // 4), (quarter + 1) * (F // 4))
                for c in range(n_ct):
                    nc.sync.dma_start(
                        out=w1_sb[c][:, fs], in_=w1[c * 128 : (c + 1) * 128, fs]
                    )
            b1_sb = wpool.tile([128, F // 128], F32, name="b1sb", tag="b1sb")
            nc.sync.dma_start(out=b1_sb, in_=b1t[:, :])
            b2_sb = wpool.tile([128, C], F32, name="b2sb", tag="b2sb")
            nc.sync.dma_start(out=b2_sb, in_=b2b[:, :])
            w2_sb = []
            for f in range(n_ft):
                t = wpool.tile([128, C], BF16, name=f"w2sb{f}", tag=f"w2sb{f}")
                nc.sync.dma_start(out=t, in_=w2[f * 128 : (f + 1) * 128, :])
                w2_sb.append(t)

            tok0 = 0
            for tk, ch in enumerate(chunks):
                if tk == 0:
                    xts = xts0
                else:
                    xts = []
                    for c in range(n_ct):
                        t = xpool.tile([128, ch], BF16, name=f"xt_{tk}_{c}", tag=f"xt{c}")
                        nc.sync.dma_start(
                            out=t,
                            in_=xt[c * 128 : (c + 1) * 128, tok0 : tok0 + ch],
                        )
                        xts.append(t)

                hts = []
                for f in range(n_ft):
                    ph = phpool.tile([128, ch], F32, name=f"ph_{tk}_{f}", tag="ph")
                    for c in range(n_ct):
                        nc.tensor.matmul(
                            ph,
                            lhsT=w1_sb[c][:, f * 128 : (f + 1) * 128],
                            rhs=xts[c],
                            start=(c == 0),
                            stop=(c == n_ct - 1),
                        )
                    ht = hpool.tile([128, ch], BF16, name=f"ht_{tk}_{f}", tag="ht")
                    nc.scalar.activation(
                        out=ht,
                        in_=ph,
                        func=mybir.ActivationFunctionType.Gelu,
                        bias=b1_sb[:, f : f + 1],
                        scale=1.0,
                    )
                    hts.append(ht)

                for tt in range((ch + 127) // 128):
                    tw = min(128, ch - tt * 128)
                    for cc in range(n_cc):
                        po = popool.tile(
                            [128, 512], F32, name=f"po_{tk}_{tt}_{cc}", tag="po"
                        )
                        for f in range(n_ft):
                            nc.tensor.matmul(
                                po[:tw, :],
                                lhsT=hts[f][:, tt * 128 : tt * 128 + tw],
                                rhs=w2_sb[f][:, cc * 512 : (cc + 1) * 512],
                                start=(f == 0),
                                stop=(f == n_ft - 1),
                            )
                        ot = opool.tile(
                            [128, 512], F32, name=f"ot_{tk}_{tt}_{cc}", tag="ot"
                        )
                        nc.vector.tensor_add(
                            ot[:tw, :], po[:tw, :], b2_sb[:tw, cc * 512 : (cc + 1) * 512]
                        )
                        r0 = tok0 + tt * 128
                        nc.sync.dma_start(
                            out=out[r0 : r0 + tw, cc * 512 : (cc + 1) * 512],
                            in_=ot[:tw, :],
                        )
                tok0 += ch
    nc.finalize()
    return nc


def pick_chunks(max_n: int) -> list[int]:
    """[512]*a + [tail]: most chunks at 512 (best PE efficiency), one exact
    tail chunk — matmul N and output partition width don't need alignment."""
    n512 = max_n // 512
    rem = max_n - n512 * 512
    chunks = [512] * n512
    if rem > 0:
        chunks.append(rem)
    if not chunks:
        chunks = [1]
    return chunks


def _route(x2d: np.ndarray, Wg: np.ndarray):
    """fp32 gate identical in selection to the reference; returns per-expert
    token indices and renormalized top-2 weights."""
    logits = x2d @ Wg  # fp32 BLAS
    order = np.argsort(-logits, axis=1, kind="stable")
    top2 = order[:, :K]  # [N, 2]
    m = logits.max(axis=1, keepdims=True)
    p = np.exp(logits - m, dtype=np.float32)
    p /= p.sum(axis=1, keepdims=True)
    tw = np.take_along_axis(p, top2, axis=1)
    tw /= tw.sum(axis=1, keepdims=True)  # [N, 2] renormalized
    idxs, ws = [], []
    for e in range(E):
        sel = top2 == e  # [N, 2] bool, at most one True per row
        rows = np.where(sel.any(axis=1))[0]
        idxs.append(rows)
        ws.append(tw[rows][sel[rows]])
    return idxs, ws


_LAST_RESULTS = {}  # stash for test harness introspection (exec time etc.)


def kernel(**inputs: np.ndarray) -> np.ndarray:
    x = np.asarray(inputs["x"], dtype=np.float32)
    Wg = np.asarray(inputs["Wg"], dtype=np.float32)
    W1 = np.asarray(inputs["W1"], dtype=np.float32)
    b1 = np.asarray(inputs["b1"], dtype=np.float32)
    W2 = np.asarray(inputs["W2"], dtype=np.float32)
    b2 = np.asarray(inputs["b2"], dtype=np.float32)

    B, T, Cx = x.shape
    assert Cx == C
    x2d = np.ascontiguousarray(x.reshape(-1, C))
    n_tok_total = x2d.shape[0]

    idxs, ws = _route(x2d, Wg)
    max_n = max(len(i) for i in idxs)
    chunks = pick_chunks(max_n)
    ntok = sum(chunks)

    in_maps = []
    for e in range(E):
        n_e = len(idxs[e])
        xe = np.zeros((ntok, C), dtype=np.float32)
        xe[:n_e] = x2d[idxs[e]]
        in_maps.append(
            {
                "xt": np.ascontiguousarray(xe.T).astype(ml_dtypes.bfloat16),
                "w1": W1[e].astype(ml_dtypes.bfloat16),
                "w2": W2[e].astype(ml_dtypes.bfloat16),
                "b1t": np.ascontiguousarray(
                    b1[e].reshape(F // 128, 128).T
                ).astype(np.float32),
                "b2b": np.broadcast_to(b2[e], (128, C)).copy().astype(np.float32),
            }
        )

    nc = build_nc(chunks)
    trace = os.environ.get("KERNEL_TRACE", "") == "1"
    res = run_bass_kernel_spmd(
        nc, in_maps, core_ids=list(range(N_CORES)), trace=trace
    )
    _LAST_RESULTS["bass_results"] = res
    if trace and res.exec_time_ns is not None:
        print(f"[kernel] HW exec time: {res.exec_time_ns} ns")

    out = np.zeros((n_tok_total, C), dtype=np.float32)
    for e in range(E):
        n_e = len(idxs[e])
        oe = np.asarray(res.results[e]["out"])[:n_e]
        out[idxs[e]] += ws[e][:, None] * oe
    return out.reshape(B, T, C)


# revision 14
# speedup vs baseline: 1.1987x; 1.0403x over previous
"""Trainium2 Bass kernel for an 8-expert top-2 MoE layer (B=4, T=2048, C=1024,
F=4096), expert-parallel across 8 NeuronCores.

Strategy
--------
The reference module is a *dense* MoE: it runs every expert's FFN on every
token, then combines with top-2 gate weights — so 6 of 8 expert outputs per
token are multiplied by zero.  The output only depends on each token's top-2
experts, so we route: the host computes the (tiny) gate in fp32, assigns each
token to its two experts, and each NeuronCore runs one expert's FFN over just
the tokens routed to it (~2*BT/E tokens, padded to a common multiple of 512).
The host then scatter-adds the gate-weighted per-expert outputs.

The gate MUST be computed in fp32: the smallest 2nd-vs-3rd expert logit margin
over the 8192 tokens is ~3.6e-5, and a bf16 gate flips the selected expert set
for ~17 tokens, each flip producing an O(1) relative error at that token.  The
fp32 host gate matches the reference selection with a ~20x margin.

On-device math per core (expert e):
    hT[f, t]   = sum_c W1[c, f] * xT[c, t]        (PE, bf16 inputs, fp32 acc)
    hT         = gelu_erf(hT + b1)                (ScalarE, fused bias)
    out[t, cc] = sum_f h[t, f] * W2[f, cc]        (PE, bf16 h, fp32 acc)
    out        = out + b2                         (VectorE, fp32)
Computing h in transposed form (tokens in the free dim) is what lets the
second matmul contract over F without any on-device transpose.
"""

import os

import numpy as np
import ml_dtypes

import concourse.bass as bass
import concourse.mybir as mybir
import concourse.tile as tile
from concourse import bacc
from concourse.bass_utils import run_bass_kernel_spmd

C = 1024
F = 4096
E = 8
K = 2
N_CORES = 8
CHUNK = 512  # tokens per device-side pipeline chunk

BF16 = mybir.dt.bfloat16
F32 = mybir.dt.float32


def build_nc(chunks: list[int]) -> bass.Bass:
    """Bass program for one expert's FFN over sum(chunks) tokens.

    chunks: per-pipeline-chunk token counts, each a multiple of 128, <= 512.
    """
    ntok = sum(chunks)
    assert all(0 < ch <= 512 for ch in chunks)
    nc = bacc.Bacc(None)

    xt = nc.dram_tensor("xt", [C, ntok], BF16, kind="ExternalInput")
    w1 = nc.dram_tensor("w1", [C, F], BF16, kind="ExternalInput")
    w2 = nc.dram_tensor("w2", [F, C], BF16, kind="ExternalInput")
    b1t = nc.dram_tensor("b1t", [128, F // 128], F32, kind="ExternalInput")
    b2b = nc.dram_tensor("b2b", [128, C], F32, kind="ExternalInput")
    out = nc.dram_tensor("out", [ntok, C], F32, kind="ExternalOutput")

    n_ct = C // 128  # 8 contraction tiles for x @ W1
    n_ft = F // 128  # 32 F tiles / contraction tiles for h @ W2
    n_cc = C // 512  # 2 output column chunks

    with tile.TileContext(nc) as tc:
        with (
            tc.tile_pool(name="wpool", bufs=1) as wpool,
            tc.tile_pool(name="xpool", bufs=2) as xpool,
            tc.tile_pool(name="hpool", bufs=n_ft + 2) as hpool,
            tc.tile_pool(name="opool", bufs=4) as opool,
            tc.tile_pool(name="phpool", bufs=4, space="PSUM") as phpool,
            tc.tile_pool(name="popool", bufs=4, space="PSUM") as popool,
        ):
            # DMA issue order matters: biases first (tiny, and the first gelu
            # blocks PSUM recycling on b1), then chunk-0 activations so the PE
            # can start ASAP, then W1 in quarters (f-tile order), then W2
            # which is not needed until the first mm2 (~75us in).
            b1_sb = wpool.tile([128, F // 128], F32, name="b1sb", tag="b1sb")
            nc.sync.dma_start(out=b1_sb, in_=b1t[:, :])
            b2_sb = wpool.tile([128, C], F32, name="b2sb", tag="b2sb")
            nc.sync.dma_start(out=b2_sb, in_=b2b[:, :])
            xts0 = []
            for c in range(n_ct):
                t = xpool.tile([128, chunks[0]], BF16, name=f"xt_0_{c}", tag=f"xt{c}")
                nc.sync.dma_start(out=t, in_=xt[c * 128 : (c + 1) * 128, : chunks[0]])
                xts0.append(t)
            w1_sb = []
            for c in range(n_ct):
                t = wpool.tile([128, F], BF16, name=f"w1sb{c}", tag=f"w1sb{c}")
                w1_sb.append(t)
            for quarter in range(4):
                fs = slice(quarter * (F // 4), (quarter + 1) * (F // 4))
                for c in range(n_ct):
                    nc.sync.dma_start(
                        out=w1_sb[c][:, fs], in_=w1[c * 128 : (c + 1) * 128, fs]
                    )
            w2_sb = []
            for f in range(n_ft):
                t = wpool.tile([128, C], BF16, name=f"w2sb{f}", tag=f"w2sb{f}")
                nc.sync.dma_start(out=t, in_=w2[f * 128 : (f + 1) * 128, :])
                w2_sb.append(t)

            tok0 = 0
            for tk, ch in enumerate(chunks):
                if tk == 0:
                    xts = xts0
                else:
                    xts = []
                    for c in range(n_ct):
                        t = xpool.tile([128, ch], BF16, name=f"xt_{tk}_{c}", tag=f"xt{c}")
                        nc.sync.dma_start(
                            out=t,
                            in_=xt[c * 128 : (c + 1) * 128, tok0 : tok0 + ch],
                        )
                        xts.append(t)

                hts = []
                for f in range(n_ft):
                    ph = phpool.tile([128, ch], F32, name=f"ph_{tk}_{f}", tag="ph")
                    for c in range(n_ct):
                        nc.tensor.matmul(
                            ph,
                            lhsT=w1_sb[c][:, f * 128 : (f + 1) * 128],
                            rhs=xts[c],
                            start=(c == 0),
                            stop=(c == n_ct - 1),
                        )
                    ht = hpool.tile([128, ch], BF16, name=f"ht_{tk}_{f}", tag="ht")
                    nc.scalar.activation(
                        out=ht,
                        in_=ph,
                        func=mybir.ActivationFunctionType.Gelu,
                        bias=b1_sb[:, f : f + 1],
                        scale=1.0,
                    )
                    hts.append(ht)

                for tt in range((ch + 127) // 128):
                    tw = min(128, ch - tt * 128)
                    for cc in range(n_cc):
                        po = popool.tile(
                            [128, 512], F32, name=f"po_{tk}_{tt}_{cc}", tag="po"
                        )
                        for f in range(n_ft):
                            nc.tensor.matmul(
                                po[:tw, :],
                                lhsT=hts[f][:, tt * 128 : tt * 128 + tw],
                                rhs=w2_sb[f][:, cc * 512 : (cc + 1) * 512],
                                start=(f == 0),
                                stop=(f == n_ft - 1),
                            )
                        ot = opool.tile(
                            [128, 512], F32, name=f"ot_{tk}_{tt}_{cc}", tag="ot"
                        )
                        nc.vector.tensor_add(
                            ot[:tw, :], po[:tw, :], b2_sb[:tw, cc * 512 : (cc + 1) * 512]
                        )
                        r0 = tok0 + tt * 128
                        nc.sync.dma_start(
                            out=out[r0 : r0 + tw, cc * 512 : (cc + 1) * 512],
                            in_=ot[:tw, :],
                        )
                tok0 += ch
    nc.finalize()
    return nc


def pick_chunks(max_n: int) -> list[int]:
    """[512]*a + [tail]: most chunks at 512 (best PE efficiency), one exact
    tail chunk — matmul N and output partition width don't need alignment."""
    n512 = max_n // 512
    rem = max_n - n512 * 512
    chunks = [512] * n512
    if rem > 0:
        chunks.append(rem)
    if not chunks:
        chunks = [1]
    return chunks


def _route(x2d: np.ndarray, Wg: np.ndarray):
    """fp32 gate identical in selection to the reference; returns per-expert
    token indices and renormalized top-2 weights."""
    logits = x2d @ Wg  # fp32 BLAS
    order = np.argsort(-logits, axis=1, kind="stable")
    top2 = order[:, :K]  # [N, 2]
    m = logits.max(axis=1, keepdims=True)
    p = np.exp(logits - m, dtype=np.float32)
    p /= p.sum(axis=1, keepdims=True)
    tw = np.take_along_axis(p, top2, axis=1)
    tw /= tw.sum(axis=1, keepdims=True)  # [N, 2] renormalized
    idxs, ws = [], []
    for e in range(E):
        sel = top2 == e  # [N, 2] bool, at most one True per row
        rows = np.where(sel.any(axis=1))[0]
        idxs.append(rows)
        ws.append(tw[rows][sel[rows]])
    return idxs, ws


_LAST_RESULTS = {}  # stash for test harness introspection (exec time etc.)


def kernel(**inputs: np.ndarray) -> np.ndarray:
    x = np.asarray(inputs["x"], dtype=np.float32)
    Wg = np.asarray(inputs["Wg"], dtype=np.float32)
    W1 = np.asarray(inputs["W1"], dtype=np.float32)
    b1 = np.asarray(inputs["b1"], dtype=np.float32)
    W2 = np.asarray(inputs["W2"], dtype=np.float32)
    b2 = np.asarray(inputs["b2"], dtype=np.float32)

    B, T, Cx = x.shape
    assert Cx == C
    x2d = np.ascontiguousarray(x.reshape(-1, C))
    n_tok_total = x2d.shape[0]

    idxs, ws = _route(x2d, Wg)
    max_n = max(len(i) for i in idxs)
    chunks = pick_chunks(max_n)
    ntok = sum(chunks)

    in_maps = []
    for e in range(E):
        n_e = len(idxs[e])
        xe = np.zeros((ntok, C), dtype=np.float32)
        xe[:n_e] = x2d[idxs[e]]
        in_maps.append(
            {
                "xt": np.ascontiguousarray(xe.T).astype(ml_dtypes.bfloat16),
                "w1": W1[e].astype(ml_dtypes.bfloat16),
                "w2": W2[e].astype(ml_dtypes.bfloat16),
                "b1t": np.ascontiguousarray(
                    b1[e].reshape(F // 128, 128).T
                ).astype(np.float32),
                "b2b": np.broadcast_to(b2[e], (128, C)).copy().astype(np.float32),
            }
        )

    nc = build_nc(chunks)
    trace = os.environ.get("KERNEL_TRACE", "") == "1"
    res = run_bass_kernel_spmd(
        nc, in_maps, core_ids=list(range(N_CORES)), trace=trace
    )
    _LAST_RESULTS["bass_results"] = res
    if trace and res.exec_time_ns is not None:
        print(f"[kernel] HW exec time: {res.exec_time_ns} ns")

    out = np.zeros((n_tok_total, C), dtype=np.float32)
    for e in range(E):
        n_e = len(idxs[e])
        oe = np.asarray(res.results[e]["out"])[:n_e]
        out[idxs[e]] += ws[e][:, None] * oe
    return out.reshape(B, T, C)


# revision 17
# speedup vs baseline: 1.2183x; 1.0164x over previous
"""Trainium2 Bass kernel for an 8-expert top-2 MoE layer (B=4, T=2048, C=1024,
F=4096), expert-parallel across 8 NeuronCores.

Strategy
--------
The reference module is a *dense* MoE: it runs every expert's FFN on every
token, then combines with top-2 gate weights — so 6 of 8 expert outputs per
token are multiplied by zero.  The output only depends on each token's top-2
experts, so we route: the host computes the (tiny) gate in fp32, assigns each
token to its two experts, and the device computes each expert's FFN over just
the tokens routed to it.  The host then scatter-adds the gate-weighted
per-expert outputs.

The gate MUST be computed in fp32: the smallest 2nd-vs-3rd expert logit margin
over the 8192 tokens is ~3.6e-5, and a bf16 gate flips the selected expert set
for ~17 tokens, each flip producing an O(1) relative error at that token.  The
fp32 host gate matches the reference selection with a ~20x margin.

Load balancing: expert token counts vary (~1930..2180), and an SPMD program
pads every core to the busiest expert.  We instead pair a big expert with a
small one (sorted largest<->smallest) and split each pair's FFN across two
cores along the F axis: core 2p+h runs BOTH experts of pair p over F-half h.
Per-core work becomes (n_big + n_small)/2 full-F-equivalents, i.e. the pair
average instead of the global max.  The two cores' partial outputs (each a
full [n, C] sum over its F-half; b2 is pre-halved on the host so the halves
sum to one b2) are added on the host during the scatter.

On-device math per core (pair p, F-half h), for each expert e in the pair:
    hT[f, t]   = sum_c W1[c, f] * xT[c, t]        (PE, bf16 inputs, fp32 acc)
    hT         = gelu_erf(hT + b1[f])             (ScalarE, fused bias)
    out[t, cc] = sum_{f in half} h[t, f] * W2[f, cc]   (PE, bf16 h, fp32 acc)
    out        = out + b2/2                       (VectorE, fp32)
Computing h in transposed form (tokens in the free dim) is what lets the
second matmul contract over F without any on-device transpose.
"""

import os

import numpy as np
import ml_dtypes

import concourse.bass as bass
import concourse.mybir as mybir
import concourse.tile as tile
from concourse import bacc
from concourse.bass_utils import run_bass_kernel_spmd

C = 1024
F = 4096
FH = F // 2  # per-core F half
E = 8
K = 2
N_CORES = 8
CHUNK = 512

BF16 = mybir.dt.bfloat16
F32 = mybir.dt.float32


def build_nc(chunks_a: list[int], chunks_b: list[int]) -> bass.Bass:
    """Bass program: two experts' FFNs (F-half depth) over their token chunks.

    chunks_a/chunks_b: per-chunk token counts for expert slot A / B,
    each 0 < ch <= 512.
    """
    nta, ntb = sum(chunks_a), sum(chunks_b)
    assert all(0 < ch <= 512 for ch in chunks_a + chunks_b)
    nc = bacc.Bacc(None)

    # inputs: token stream and weights for expert slots A and B
    xta = nc.dram_tensor("xta", [C, nta], BF16, kind="ExternalInput")
    xtb = nc.dram_tensor("xtb", [C, ntb], BF16, kind="ExternalInput")
    w1a = nc.dram_tensor("w1a", [C, FH], BF16, kind="ExternalInput")
    w1b = nc.dram_tensor("w1b", [C, FH], BF16, kind="ExternalInput")
    w2a = nc.dram_tensor("w2a", [FH, C], BF16, kind="ExternalInput")
    w2b = nc.dram_tensor("w2b", [FH, C], BF16, kind="ExternalInput")
    # b1t[s][p, j] = b1[slot s][(j*128)+p] for this core's F-half (j: f-tile)
    b1t = nc.dram_tensor("b1t", [2, 128, FH // 128], F32, kind="ExternalInput")
    # b2h[s] = b2[slot s] / 2, broadcast over partitions on device
    b2h = nc.dram_tensor("b2h", [2, C], F32, kind="ExternalInput")
    outa = nc.dram_tensor("outa", [nta, C], F32, kind="ExternalOutput")
    outb = nc.dram_tensor("outb", [ntb, C], F32, kind="ExternalOutput")

    n_ct = C // 128  # 8 contraction tiles for x @ W1
    n_ft = FH // 128  # 16 F tiles per half
    n_cc = C // 512  # 2 output column chunks

    with tile.TileContext(nc) as tc:
        with (
            tc.tile_pool(name="wpool", bufs=1) as wpool,
            tc.tile_pool(name="xpool", bufs=2) as xpool,
            tc.tile_pool(name="hpool", bufs=n_ft + 2) as hpool,
            tc.tile_pool(name="opool", bufs=4) as opool,
            tc.tile_pool(name="phpool", bufs=4, space="PSUM") as phpool,
            tc.tile_pool(name="popool", bufs=4, space="PSUM") as popool,
        ):
            # DMA issue order: biases first (tiny; the first gelu blocks PSUM
            # recycling on b1), chunk-0 activations, W1a in quarters (f-tile
            # order), then W2a / slot-B tensors which are needed later.
            b1_sb = wpool.tile([128, 2, n_ft], F32, name="b1sb", tag="b1sb")
            nc.sync.dma_start(out=b1_sb[:, 0, :], in_=b1t[0])
            nc.sync.dma_start(out=b1_sb[:, 1, :], in_=b1t[1])
            b2_sb = wpool.tile([128, 2, C], F32, name="b2sb", tag="b2sb")
            for s in range(2):
                nc.sync.dma_start(
                    out=b2_sb[:, s, :], in_=b2h[s : s + 1, :].to_broadcast([128, C])
                )

            xts0 = []
            for c in range(n_ct):
                t = xpool.tile([128, chunks_a[0]], BF16, name=f"xta_0_{c}", tag=f"xt{c}")
                nc.sync.dma_start(out=t, in_=xta[c * 128 : (c + 1) * 128, : chunks_a[0]])
                xts0.append(t)

            w1_sb = {}  # slot -> list of c-tiles [128, FH]
            w2_sb = {}  # slot -> list of f-tiles [128, C]
            for s, w1d in ((0, w1a), (1, w1b)):
                w1_sb[s] = [
                    wpool.tile([128, FH], BF16, name=f"w1sb{s}_{c}", tag=f"w1sb{s}_{c}")
                    for c in range(n_ct)
                ]
                if s == 0:
                    for quarter in range(4):
                        fs = slice(quarter * (FH // 4), (quarter + 1) * (FH // 4))
                        for c in range(n_ct):
                            nc.sync.dma_start(
                                out=w1_sb[s][c][:, fs],
                                in_=w1d[c * 128 : (c + 1) * 128, fs],
                            )
                else:
                    for c in range(n_ct):
                        nc.sync.dma_start(
                            out=w1_sb[s][c], in_=w1d[c * 128 : (c + 1) * 128, :]
                        )
            for s, w2d in ((0, w2a), (1, w2b)):
                w2_sb[s] = []
                for f in range(n_ft):
                    t = wpool.tile([128, C], BF16, name=f"w2sb{s}_{f}", tag=f"w2sb{s}_{f}")
                    nc.sync.dma_start(out=t, in_=w2d[f * 128 : (f + 1) * 128, :])
                    w2_sb[s].append(t)

            def run_slot(s, xtd, outd, chunks, first_xts):
                tok0 = 0
                for tk, ch in enumerate(chunks):
                    if first_xts is not None and tk == 0:
                        xts = first_xts
                    else:
                        xts = []
                        for c in range(n_ct):
                            t = xpool.tile(
                                [128, ch], BF16, name=f"xt{s}_{tk}_{c}", tag=f"xt{c}"
                            )
                            nc.sync.dma_start(
                                out=t,
                                in_=xtd[c * 128 : (c + 1) * 128, tok0 : tok0 + ch],
                            )
                            xts.append(t)

                    hts = []
                    for f in range(n_ft):
                        ph = phpool.tile([128, ch], F32, name=f"ph{s}_{tk}_{f}", tag="ph")
                        for c in range(n_ct):
                            nc.tensor.matmul(
                                ph,
                                lhsT=w1_sb[s][c][:, f * 128 : (f + 1) * 128],
                                rhs=xts[c],
                                start=(c == 0),
                                stop=(c == n_ct - 1),
                            )
                        ht = hpool.tile([128, ch], BF16, name=f"ht{s}_{tk}_{f}", tag="ht")
                        nc.scalar.activation(
                            out=ht,
                            in_=ph,
                            func=mybir.ActivationFunctionType.Gelu,
                            bias=b1_sb[:, s, f : f + 1],
                            scale=1.0,
                        )
                        hts.append(ht)

                    for tt in range((ch + 127) // 128):
                        tw = min(128, ch - tt * 128)
                        for cc in range(n_cc):
                            po = popool.tile(
                                [128, 512], F32, name=f"po{s}_{tk}_{tt}_{cc}", tag="po"
                            )
                            for f in range(n_ft):
                                nc.tensor.matmul(
                                    po[:tw, :],
                                    lhsT=hts[f][:, tt * 128 : tt * 128 + tw],
                                    rhs=w2_sb[s][f][:, cc * 512 : (cc + 1) * 512],
                                    start=(f == 0),
                                    stop=(f == n_ft - 1),
                                )
                            ot = opool.tile(
                                [128, 512], F32, name=f"ot{s}_{tk}_{tt}_{cc}", tag="ot"
                            )
                            nc.vector.tensor_add(
                                ot[:tw, :],
                                po[:tw, :],
                                b2_sb[:tw, s, cc * 512 : (cc + 1) * 512],
                            )
                            r0 = tok0 + tt * 128
                            nc.sync.dma_start(
                                out=outd[r0 : r0 + tw, cc * 512 : (cc + 1) * 512],
                                in_=ot[:tw, :],
                            )
                    tok0 += ch

            run_slot(0, xta, outa, chunks_a, xts0)
            run_slot(1, xtb, outb, chunks_b, None)
    nc.finalize()
    return nc


def pick_chunks(n: int) -> list[int]:
    """[512]*a + [exact tail] — matmul N needs no alignment."""
    n512 = n // 512
    rem = n - n512 * 512
    chunks = [512] * n512
    if rem > 0:
        chunks.append(rem)
    if not chunks:
        chunks = [1]
    return chunks


def _route(x2d: np.ndarray, Wg: np.ndarray):
    """fp32 gate identical in selection to the reference; returns per-expert
    token indices and renormalized top-2 weights."""
    logits = x2d @ Wg  # fp32 BLAS
    order = np.argsort(-logits, axis=1, kind="stable")
    top2 = order[:, :K]  # [N, 2]
    m = logits.max(axis=1, keepdims=True)
    p = np.exp(logits - m, dtype=np.float32)
    p /= p.sum(axis=1, keepdims=True)
    tw = np.take_along_axis(p, top2, axis=1)
    tw /= tw.sum(axis=1, keepdims=True)  # [N, 2] renormalized
    idxs, ws = [], []
    for e in range(E):
        sel = top2 == e  # [N, 2] bool, at most one True per row
        rows = np.where(sel.any(axis=1))[0]
        idxs.append(rows)
        ws.append(tw[rows][sel[rows]])
    return idxs, ws


_LAST_RESULTS = {}  # stash for test harness introspection (exec time etc.)


def kernel(**inputs: np.ndarray) -> np.ndarray:
    x = np.asarray(inputs["x"], dtype=np.float32)
    Wg = np.asarray(inputs["Wg"], dtype=np.float32)
    W1 = np.asarray(inputs["W1"], dtype=np.float32)
    b1 = np.asarray(inputs["b1"], dtype=np.float32)
    W2 = np.asarray(inputs["W2"], dtype=np.float32)
    b2 = np.asarray(inputs["b2"], dtype=np.float32)

    B, T, Cx = x.shape
    assert Cx == C
    x2d = np.ascontiguousarray(x.reshape(-1, C))
    n_tok_total = x2d.shape[0]

    idxs, ws = _route(x2d, Wg)
    counts = np.array([len(i) for i in idxs])

    # Pair the largest expert with the smallest, 2nd largest with 2nd
    # smallest, etc.  Pair p runs on cores 2p (F-half 0) and 2p+1 (F-half 1).
    order = np.argsort(-counts, kind="stable")
    pairs = [(int(order[p]), int(order[E - 1 - p])) for p in range(E // 2)]
    nta = max(counts[a] for a, _ in pairs)
    ntb = max(counts[b] for _, b in pairs)
    chunks_a = pick_chunks(int(nta))
    chunks_b = pick_chunks(int(ntb))
    nta, ntb = sum(chunks_a), sum(chunks_b)

    w1h = W1.astype(ml_dtypes.bfloat16)  # [E, C, F]
    w2h = W2.astype(ml_dtypes.bfloat16)  # [E, F, C]

    def xt_for(e, ntok):
        xe = np.zeros((ntok, C), dtype=np.float32)
        xe[: counts[e]] = x2d[idxs[e]]
        return np.ascontiguousarray(xe.T).astype(ml_dtypes.bfloat16)

    xt_cache = {}
    for a, b_ in pairs:
        xt_cache[a] = xt_for(a, nta)
        xt_cache[b_] = xt_for(b_, ntb)

    in_maps = []
    for core in range(N_CORES):
        p, h = divmod(core, 2)
        ea, eb = pairs[p]
        fsl = slice(h * FH, (h + 1) * FH)
        b1t = np.stack(
            [
                np.ascontiguousarray(b1[ea][fsl].reshape(FH // 128, 128).T),
                np.ascontiguousarray(b1[eb][fsl].reshape(FH // 128, 128).T),
            ]
        ).astype(np.float32)
        in_maps.append(
            {
                "xta": xt_cache[ea],
                "xtb": xt_cache[eb],
                "w1a": np.ascontiguousarray(w1h[ea][:, fsl]),
                "w1b": np.ascontiguousarray(w1h[eb][:, fsl]),
                "w2a": np.ascontiguousarray(w2h[ea][fsl, :]),
                "w2b": np.ascontiguousarray(w2h[eb][fsl, :]),
                "b1t": b1t,
                "b2h": np.stack([b2[ea], b2[eb]]).astype(np.float32) * 0.5,
            }
        )

    nc = build_nc(chunks_a, chunks_b)
    trace = os.environ.get("KERNEL_TRACE", "") == "1"
    res = run_bass_kernel_spmd(
        nc, in_maps, core_ids=list(range(N_CORES)), trace=trace
    )
    _LAST_RESULTS["bass_results"] = res
    if trace and res.exec_time_ns is not None:
        print(f"[kernel] HW exec time: {res.exec_time_ns} ns")

    out = np.zeros((n_tok_total, C), dtype=np.float32)
    for p, (ea, eb) in enumerate(pairs):
        for e, key in ((ea, "outa"), (eb, "outb")):
            n_e = counts[e]
            oe = (
                np.asarray(res.results[2 * p][key])[:n_e]
                + np.asarray(res.results[2 * p + 1][key])[:n_e]
            )
            out[idxs[e]] += ws[e][:, None] * oe
    return out.reshape(B, T, C)


# revision 21
# speedup vs baseline: 1.2492x; 1.0253x over previous
"""Trainium2 Bass kernel for an 8-expert top-2 MoE layer (B=4, T=2048, C=1024,
F=4096), expert-parallel across 8 NeuronCores.

Strategy
--------
The reference module is a *dense* MoE: it runs every expert's FFN on every
token, then combines with top-2 gate weights — so 6 of 8 expert outputs per
token are multiplied by zero.  The output only depends on each token's top-2
experts, so we route: the host computes the (tiny) gate in fp32, assigns each
token to its two experts, and the device computes each expert's FFN over just
the tokens routed to it.  The host then scatter-adds the gate-weighted
per-expert outputs.

The gate MUST be computed in fp32: the smallest 2nd-vs-3rd expert logit margin
over the 8192 tokens is ~3.6e-5, and a bf16 gate flips the selected expert set
for ~17 tokens, each flip producing an O(1) relative error at that token.  The
fp32 host gate matches the reference selection with a ~20x margin.

Load balancing: expert token counts vary (~1930..2180), and an SPMD program
pads every core to the busiest expert.  We instead pair a big expert with a
small one (sorted largest<->smallest) and split each pair's FFN across two
cores along the F axis: core 2p+h runs BOTH experts of pair p over F-half h.
Per-core work becomes (n_big + n_small)/2 full-F-equivalents, i.e. the pair
average instead of the global max.  The two cores' partial outputs (each a
full [n, C] sum over its F-half; b2 is pre-halved on the host so the halves
sum to one b2) are added on the host during the scatter.

On-device math per core (pair p, F-half h), for each expert e in the pair:
    hT[f, t]   = sum_c W1[c, f] * xT[c, t]        (PE, bf16 inputs, fp32 acc)
    hT         = gelu_erf(hT + b1[f])             (ScalarE, fused bias)
    out[t, cc] = sum_{f in half} h[t, f] * W2[f, cc]   (PE, bf16 h, fp32 acc)
    out        = out + b2/2                       (VectorE, fp32)
Computing h in transposed form (tokens in the free dim) is what lets the
second matmul contract over F without any on-device transpose.
"""

import os

import numpy as np
import ml_dtypes

import concourse.bass as bass
import concourse.mybir as mybir
import concourse.tile as tile
from concourse import bacc
from concourse.bass_utils import run_bass_kernel_spmd

C = 1024
F = 4096
FH = F // 2  # per-core F half
E = 8
K = 2
N_CORES = 8
CHUNK = 512

BF16 = mybir.dt.bfloat16
F32 = mybir.dt.float32


def build_nc(chunks_a: list[int], chunks_b: list[int]) -> bass.Bass:
    """Bass program: two experts' FFNs (F-half depth) over their token chunks.

    chunks_a/chunks_b: per-chunk token counts for expert slot A / B,
    each 0 < ch <= 512.
    """
    nta, ntb = sum(chunks_a), sum(chunks_b)
    assert all(0 < ch <= 512 for ch in chunks_a + chunks_b)
    nc = bacc.Bacc(None)

    # inputs: token stream and weights for expert slots A and B
    xta = nc.dram_tensor("xta", [C, nta], BF16, kind="ExternalInput")
    xtb = nc.dram_tensor("xtb", [C, ntb], BF16, kind="ExternalInput")
    w1a = nc.dram_tensor("w1a", [C, FH], BF16, kind="ExternalInput")
    w1b = nc.dram_tensor("w1b", [C, FH], BF16, kind="ExternalInput")
    w2a = nc.dram_tensor("w2a", [FH, C], BF16, kind="ExternalInput")
    w2b = nc.dram_tensor("w2b", [FH, C], BF16, kind="ExternalInput")
    # b1t[s][p, j] = b1[slot s][(j*128)+p] for this core's F-half (j: f-tile)
    b1t = nc.dram_tensor("b1t", [2, 128, FH // 128], F32, kind="ExternalInput")
    # b2h[s] = b2[slot s] / 2, broadcast over partitions on device
    b2h = nc.dram_tensor("b2h", [2, C], F32, kind="ExternalInput")
    outa = nc.dram_tensor("outa", [nta, C], F32, kind="ExternalOutput")
    outb = nc.dram_tensor("outb", [ntb, C], F32, kind="ExternalOutput")

    n_ct = C // 128  # 8 contraction tiles for x @ W1
    n_ft = FH // 128  # 16 F tiles per half
    n_cc = C // 512  # 2 output column chunks

    with tile.TileContext(nc) as tc:
        with (
            tc.tile_pool(name="wpool", bufs=1) as wpool,
            tc.tile_pool(name="xpool", bufs=3) as xpool,
            tc.tile_pool(name="hpool", bufs=n_ft + 2) as hpool,
            tc.tile_pool(name="opool", bufs=4) as opool,
            tc.tile_pool(name="phpool", bufs=4, space="PSUM") as phpool,
            tc.tile_pool(name="popool", bufs=4, space="PSUM") as popool,
        ):
            # DMA issue order: biases first (tiny; the first gelu blocks PSUM
            # recycling on b1), chunk-0 activations, W1a in quarters (f-tile
            # order), then W2a / slot-B tensors which are needed later.
            b1_sb = wpool.tile([128, 2, n_ft], F32, name="b1sb", tag="b1sb")
            nc.sync.dma_start(out=b1_sb[:, 0, :], in_=b1t[0])
            nc.sync.dma_start(out=b1_sb[:, 1, :], in_=b1t[1])
            b2_sb = wpool.tile([128, 2, C], F32, name="b2sb", tag="b2sb")
            for s in range(2):
                nc.sync.dma_start(
                    out=b2_sb[:, s, :], in_=b2h[s : s + 1, :].to_broadcast([128, C])
                )

            xts0 = []
            for c in range(n_ct):
                t = xpool.tile([128, chunks_a[0]], BF16, name=f"xta_0_{c}", tag=f"xt{c}")
                nc.sync.dma_start(out=t, in_=xta[c * 128 : (c + 1) * 128, : chunks_a[0]])
                xts0.append(t)

            w1_sb = {
                s: [
                    wpool.tile([128, FH], BF16, name=f"w1sb{s}_{c}", tag=f"w1sb{s}_{c}")
                    for c in range(n_ct)
                ]
                for s in range(2)
            }
            w2_sb = {
                s: [
                    wpool.tile([128, C], BF16, name=f"w2sb{s}_{f}", tag=f"w2sb{s}_{f}")
                    for f in range(n_ft)
                ]
                for s in range(2)
            }
            # slot-A weights up front: W1a in quarters (f-tile order), then W2a
            for quarter in range(4):
                fs = slice(quarter * (FH // 4), (quarter + 1) * (FH // 4))
                for c in range(n_ct):
                    nc.sync.dma_start(
                        out=w1_sb[0][c][:, fs], in_=w1a[c * 128 : (c + 1) * 128, fs]
                    )
            for f in range(n_ft):
                nc.sync.dma_start(out=w2_sb[0][f], in_=w2a[f * 128 : (f + 1) * 128, :])

            # slot-B weight loads, spread between slot-A chunks so they don't
            # starve the slot-A activation streams in the DMA queues.
            deferred_loads = [
                [
                    lambda c=c: nc.sync.dma_start(
                        out=w1_sb[1][c], in_=w1b[c * 128 : (c + 1) * 128, :]
                    )
                    for c in range(n_ct)
                ],
                [
                    lambda f=f: nc.sync.dma_start(
                        out=w2_sb[1][f], in_=w2b[f * 128 : (f + 1) * 128, :]
                    )
                    for f in range(n_ft // 2)
                ],
                [
                    lambda f=f: nc.sync.dma_start(
                        out=w2_sb[1][f], in_=w2b[f * 128 : (f + 1) * 128, :]
                    )
                    for f in range(n_ft // 2, n_ft)
                ],
            ]

            def run_slot(s, xtd, outd, chunks, first_xts):
                tok0 = 0
                for tk, ch in enumerate(chunks):
                    if first_xts is not None and tk == 0:
                        xts = first_xts
                    else:
                        xts = []
                        for c in range(n_ct):
                            t = xpool.tile(
                                [128, ch], BF16, name=f"xt{s}_{tk}_{c}", tag=f"xt{c}"
                            )
                            nc.sync.dma_start(
                                out=t,
                                in_=xtd[c * 128 : (c + 1) * 128, tok0 : tok0 + ch],
                            )
                            xts.append(t)
                    if s == 0 and tk >= 1 and deferred_loads:
                        for emit in deferred_loads.pop(0):
                            emit()

                    hts = []
                    for f in range(n_ft):
                        ph = phpool.tile([128, ch], F32, name=f"ph{s}_{tk}_{f}", tag="ph")
                        for c in range(n_ct):
                            nc.tensor.matmul(
                                ph,
                                lhsT=w1_sb[s][c][:, f * 128 : (f + 1) * 128],
                                rhs=xts[c],
                                start=(c == 0),
                                stop=(c == n_ct - 1),
                            )
                        ht = hpool.tile([128, ch], BF16, name=f"ht{s}_{tk}_{f}", tag="ht")
                        nc.scalar.activation(
                            out=ht,
                            in_=ph,
                            func=mybir.ActivationFunctionType.Gelu,
                            bias=b1_sb[:, s, f : f + 1],
                            scale=1.0,
                        )
                        hts.append(ht)

                    for tt in range((ch + 127) // 128):
                        tw = min(128, ch - tt * 128)
                        for cc in range(n_cc):
                            po = popool.tile(
                                [128, 512], F32, name=f"po{s}_{tk}_{tt}_{cc}", tag="po"
                            )
                            for f in range(n_ft):
                                nc.tensor.matmul(
                                    po[:tw, :],
                                    lhsT=hts[f][:, tt * 128 : tt * 128 + tw],
                                    rhs=w2_sb[s][f][:, cc * 512 : (cc + 1) * 512],
                                    start=(f == 0),
                                    stop=(f == n_ft - 1),
                                )
                            ot = opool.tile(
                                [128, 512], F32, name=f"ot{s}_{tk}_{tt}_{cc}", tag="ot"
                            )
                            nc.vector.tensor_add(
                                ot[:tw, :],
                                po[:tw, :],
                                b2_sb[:tw, s, cc * 512 : (cc + 1) * 512],
                            )
                            r0 = tok0 + tt * 128
                            nc.sync.dma_start(
                                out=outd[r0 : r0 + tw, cc * 512 : (cc + 1) * 512],
                                in_=ot[:tw, :],
                            )
                    tok0 += ch

            run_slot(0, xta, outa, chunks_a, xts0)
            while deferred_loads:  # in case slot A had very few chunks
                for emit in deferred_loads.pop(0):
                    emit()
            run_slot(1, xtb, outb, chunks_b, None)
    nc.finalize()
    return nc


def pick_chunks(n: int) -> list[int]:
    """[512]*a + [exact tail] — matmul N needs no alignment."""
    n512 = n // 512
    rem = n - n512 * 512
    chunks = [512] * n512
    if rem > 0:
        chunks.append(rem)
    if not chunks:
        chunks = [1]
    return chunks


def _route(x2d: np.ndarray, Wg: np.ndarray):
    """fp32 gate identical in selection to the reference; returns per-expert
    token indices and renormalized top-2 weights."""
    logits = x2d @ Wg  # fp32 BLAS
    order = np.argsort(-logits, axis=1, kind="stable")
    top2 = order[:, :K]  # [N, 2]
    m = logits.max(axis=1, keepdims=True)
    p = np.exp(logits - m, dtype=np.float32)
    p /= p.sum(axis=1, keepdims=True)
    tw = np.take_along_axis(p, top2, axis=1)
    tw /= tw.sum(axis=1, keepdims=True)  # [N, 2] renormalized
    idxs, ws = [], []
    for e in range(E):
        sel = top2 == e  # [N, 2] bool, at most one True per row
        rows = np.where(sel.any(axis=1))[0]
        idxs.append(rows)
        ws.append(tw[rows][sel[rows]])
    return idxs, ws


_LAST_RESULTS = {}  # stash for test harness introspection (exec time etc.)


def kernel(**inputs: np.ndarray) -> np.ndarray:
    x = np.asarray(inputs["x"], dtype=np.float32)
    Wg = np.asarray(inputs["Wg"], dtype=np.float32)
    W1 = np.asarray(inputs["W1"], dtype=np.float32)
    b1 = np.asarray(inputs["b1"], dtype=np.float32)
    W2 = np.asarray(inputs["W2"], dtype=np.float32)
    b2 = np.asarray(inputs["b2"], dtype=np.float32)

    B, T, Cx = x.shape
    assert Cx == C
    x2d = np.ascontiguousarray(x.reshape(-1, C))
    n_tok_total = x2d.shape[0]

    idxs, ws = _route(x2d, Wg)
    counts = np.array([len(i) for i in idxs])

    # Pair the largest expert with the smallest, 2nd largest with 2nd
    # smallest, etc.  Pair p runs on cores 2p (F-half 0) and 2p+1 (F-half 1).
    order = np.argsort(-counts, kind="stable")
    pairs = [(int(order[p]), int(order[E - 1 - p])) for p in range(E // 2)]
    nta = max(counts[a] for a, _ in pairs)
    ntb = max(counts[b] for _, b in pairs)
    chunks_a = pick_chunks(int(nta))
    chunks_b = pick_chunks(int(ntb))
    nta, ntb = sum(chunks_a), sum(chunks_b)

    w1h = W1.astype(ml_dtypes.bfloat16)  # [E, C, F]
    w2h = W2.astype(ml_dtypes.bfloat16)  # [E, F, C]

    def xt_for(e, ntok):
        xe = np.zeros((ntok, C), dtype=np.float32)
        xe[: counts[e]] = x2d[idxs[e]]
        return np.ascontiguousarray(xe.T).astype(ml_dtypes.bfloat16)

    xt_cache = {}
    for a, b_ in pairs:
        xt_cache[a] = xt_for(a, nta)
        xt_cache[b_] = xt_for(b_, ntb)

    in_maps = []
    for core in range(N_CORES):
        p, h = divmod(core, 2)
        ea, eb = pairs[p]
        fsl = slice(h * FH, (h + 1) * FH)
        b1t = np.stack(
            [
                np.ascontiguousarray(b1[ea][fsl].reshape(FH // 128, 128).T),
                np.ascontiguousarray(b1[eb][fsl].reshape(FH // 128, 128).T),
            ]
        ).astype(np.float32)
        in_maps.append(
            {
                "xta": xt_cache[ea],
                "xtb": xt_cache[eb],
                "w1a": np.ascontiguousarray(w1h[ea][:, fsl]),
                "w1b": np.ascontiguousarray(w1h[eb][:, fsl]),
                "w2a": np.ascontiguousarray(w2h[ea][fsl, :]),
                "w2b": np.ascontiguousarray(w2h[eb][fsl, :]),
                "b1t": b1t,
                "b2h": np.stack([b2[ea], b2[eb]]).astype(np.float32) * 0.5,
            }
        )

    nc = build_nc(chunks_a, chunks_b)
    trace = os.environ.get("KERNEL_TRACE", "") == "1"
    res = run_bass_kernel_spmd(
        nc, in_maps, core_ids=list(range(N_CORES)), trace=trace
    )
    _LAST_RESULTS["bass_results"] = res
    if trace and res.exec_time_ns is not None:
        print(f"[kernel] HW exec time: {res.exec_time_ns} ns")

    out = np.zeros((n_tok_total, C), dtype=np.float32)
    for p, (ea, eb) in enumerate(pairs):
        for e, key in ((ea, "outa"), (eb, "outb")):
            n_e = counts[e]
            oe = (
                np.asarray(res.results[2 * p][key])[:n_e]
                + np.asarray(res.results[2 * p + 1][key])[:n_e]
            )
            out[idxs[e]] += ws[e][:, None] * oe
    return out.reshape(B, T, C)
